# revision 22
# baseline (speedup 1.0000x reference)
"""Trainium2 Bass kernel for the 3-layer GAT denoising model
(nn_Denoising_Model_24764781429262): N=50000 nodes, E=800000 edges, 8 heads.

Design:
- bf16 table [NROWS, 384]: H in c-major layout (cols 0:256), ones cols
  256:264 (softmax denominator folded into the main weighted reduce),
  alpha_src 264:272, alpha_dst 272:280. 768B gather rows.
- Replicated dense: every core computes the FULL table with PE matmuls from
  an AllGather'd transposed h (bf16) -- layer 0 needs NO collective at all
  (h0 is host-replicated input).
- Collectives tapered-chunked per layer (big chunks early so they hide
  under the edge phase, small final chunk to minimize the exposed tail)
  and issued from the PE queue so they never block the gather-critical
  Pool queue; deep hfull buffering lets DVE run ahead during the block.
- Greedy balanced A/B source split minimizes ELL padding; self-loop edges
  are forced to slot 0 of their half so the own-tile alpha_dst comes from
  the main gathers (mask-select) with no extra gather instructions.
- Edge phase fully bf16 (packed 2x DVE); weighted slot-sum via in-place
  halving tree, final combine f32.

kernel(**inputs) takes full unsharded inputs, returns full [50000, 8] f32.
"""

import math
import os
import numpy as np
import ml_dtypes

os.environ.setdefault("NEURON_RT_RESET_CORES", "1")

import concourse.bacc as bacc
import concourse.bass as cbass
import concourse.mybir as mybir
import concourse.tile as tile
from concourse.masks import make_identity

bf16 = ml_dtypes.bfloat16
N_CORES = 8
NT = 49
L = 128 * NT
PB = L + 8
NROWS = N_CORES * PB
HALF = 4 * PB
NSLOT = N_CORES * L
HS = NSLOT // 2
NH = 8
HC = 256
CT = 384          # table row elements (bf16) = 768 B
CW = 280          # table cols actually computed/written
F0 = 144          # layer-0 h rows (x 0:128 | qY 128:136 | one 136 | pad)
FH = 272          # layer 1/2 h rows (Hcm 0:256 | qY 256:264 | one 264 | pad)
CMAX = 20
F32 = mybir.dt.float32
BF16 = mybir.dt.bfloat16
I16 = mybir.dt.int16
AF = mybir.ActivationFunctionType
OP = mybir.AluOpType

# c-major permutation: cm col j holds std channel (j%8)*32 + j//8
_J = np.arange(HC)
STDCOL = (_J % 8) * 32 + _J // 8

# AllGather/dense pipeline chunk boundaries (tiles); tapered so the final
# exposed chunk is small.
CHB = [0, 16, 32, 44, NT]
AG_ENGINE = os.environ.get("AG_ENGINE", "pool")


def chunk_of_tile(i):
    for c in range(len(CHB) - 1):
        if i < CHB[c + 1]:
            return c
    raise AssertionError


# ----------------------------------------------------------------------------
# host preprocessing
# ----------------------------------------------------------------------------
def greedy_half(src, dst, n):
    order = np.argsort(src, kind="stable")
    ds = dst[order]
    counts = np.bincount(src, minlength=n)
    starts = np.r_[0, np.cumsum(counts)]
    imb = np.zeros(n, np.int32)
    half = np.zeros(n, bool)
    capA = capB = 0
    for s in np.argsort(-counts, kind="stable"):
        nb = ds[starts[s]:starts[s + 1]]
        sc = int(imb[nb].sum())
        toB = sc > 0 or (sc == 0 and capB < capA)
        if toB and capB >= HS:
            toB = False
        if (not toB) and capA >= HS:
            toB = True
        if toB:
            imb[nb] -= 1
            capB += 1
            half[s] = True
        else:
            imb[nb] += 1
            capA += 1
    return half


def preprocess(adj, n, cmax=CMAX):
    src = np.concatenate([adj[0], np.arange(n)]).astype(np.int64)
    dst = np.concatenate([adj[1], np.arange(n)]).astype(np.int64)
    half_bit = greedy_half(src, dst, n)

    src_is_b = half_bit[src]
    degA = np.bincount(dst[~src_is_b], minlength=n)
    degB = np.bincount(dst[src_is_b], minlength=n)
    deg = degA + degB

    A_nodes = np.flatnonzero(~half_bit)
    B_nodes = np.flatnonzero(half_bit)
    assert len(A_nodes) <= HS and len(B_nodes) <= HS
    A_sorted = A_nodes[np.lexsort((degA[A_nodes], deg[A_nodes]))]
    B_sorted = B_nodes[np.lexsort((degA[B_nodes], deg[B_nodes]))]
    A_list = np.concatenate([np.full(HS - len(A_sorted), -1, np.int64), A_sorted])
    B_list = np.concatenate([np.full(HS - len(B_sorted), -1, np.int64), B_sorted])

    slots = np.full(NSLOT, -1, dtype=np.int64)
    r = np.arange(NSLOT)
    t = r // 128
    k = t % N_CORES
    i = t // N_CORES
    p = r % 128
    jA = i * 4 + k
    jB = i * 4 + (k - 4)
    selA = k < 4
    slots[selA] = A_list[jA[selA] * 128 + p[selA]]
    slots[~selA] = B_list[jB[~selA] * 128 + p[~selA]]
    physrow = k * PB + i * 128 + p
    node2phys = np.full(n, -1, dtype=np.int64)
    real = slots >= 0
    node2phys[slots[real]] = physrow[real]
    assert (node2phys >= 0).all()
    assert (node2phys[A_nodes] < HALF).all()
    assert (node2phys[B_nodes] >= HALF).all()

    dphys = node2phys[dst]
    dk = dphys // PB
    dloc = dphys % PB
    di = dloc // 128
    dp = dloc % 128
    DA = np.zeros((N_CORES, NT), np.int64)
    DB = np.zeros((N_CORES, NT), np.int64)
    node_k = node2phys // PB
    node_i = (node2phys % PB) // 128
    np.maximum.at(DA, (node_k, node_i), degA)
    np.maximum.at(DB, (node_k, node_i), degB)
    DAi = np.maximum(DA.max(axis=0), 1)
    DBi = np.maximum(DB.max(axis=0), 1)

    # per-(dst, half) rank, self-loop forced to rank 0 of its half
    not_self = (src != dst).astype(np.int64)
    es = np.lexsort((src, not_self, dst))
    ds_, isb_ = dst[es], src_is_b[es]
    dk_, di_, dp_ = dk[es], di[es], dp[es]
    sphys_ = node2phys[src[es]]
    keys = ds_ * 2 + isb_.astype(np.int64)
    sort2 = np.argsort(keys, kind="stable")
    ks_ = keys[sort2]
    starts = np.r_[0, np.flatnonzero(np.diff(ks_)) + 1]
    cum = np.arange(len(ks_))
    seg_start = np.repeat(cum[starts], np.diff(np.r_[starts, len(ks_)]))
    rank = cum - seg_start
    jcol = np.empty(len(ks_), np.int64)
    jcol[sort2] = rank

    sizesA = 128 * DAi
    sizesB = 128 * DBi
    offA = np.concatenate([[0], np.cumsum(sizesA)[:-1]])
    offB = np.concatenate([[0], np.cumsum(sizesB)[:-1]])
    bigA = [np.full(int(sizesA.sum()), L, np.int32) for _ in range(N_CORES)]
    bigB = [np.full(int(sizesB.sum()), L, np.int32) for _ in range(N_CORES)]
    selB = isb_
    for kk in range(N_CORES):
        mA = (~selB) & (dk_ == kk)
        bigA[kk][offA[di_[mA]] + jcol[mA] * 128 + dp_[mA]] = sphys_[mA]
        mB = selB & (dk_ == kk)
        bigB[kk][offB[di_[mB]] + jcol[mB] * 128 + dp_[mB]] = sphys_[mB] - HALF
    coreA = [[bigA[kk][offA[ii]:offA[ii] + sizesA[ii]] for ii in range(NT)]
             for kk in range(N_CORES)]
    coreB = [[bigB[kk][offB[ii]:offB[ii] + sizesB[ii]] for ii in range(NT)]
             for kk in range(N_CORES)]

    # sanity: self-loop of dst (k,i,p) sits at col 0 of its half-block
    plan = []
    for ii in range(NT):
        ch = []
        for hf, D in (("A", int(DAi[ii])), ("B", int(DBi[ii]))):
            c0 = 0
            while c0 < D:
                cc = min(cmax, D - c0)
                ch.append((hf, c0, cc))
                c0 += cc
        plan.append(ch)

    # per-core dst-half masks [128, NT]
    maskA = np.zeros((N_CORES, 128, NT), np.float32)
    for kk in range(N_CORES):
        if kk < 4:
            maskA[kk, :, :] = 1.0
    # (dst half == core block half: cores 0..3 hold A-half dsts)

    return dict(slots=slots, node2phys=node2phys, DAi=DAi, DBi=DBi,
                coreA=coreA, coreB=coreB, plan=plan, maskA=maskA,
                half_bit=half_bit, n=n)


def wrap_idx(block_i32):
    num = block_i32.shape[0]
    assert num % 16 == 0
    g = block_i32.reshape(num // 16, 16).T.astype(np.int16)
    return np.tile(g, (8, 1))


def build_weights(inputs):
    W = [np.asarray(inputs[f"W{i}"], np.float32) for i in range(3)]
    att_src = np.asarray(inputs["att_src"], np.float32)
    att_dst = np.asarray(inputs["att_dst"], np.float32)
    bias = np.asarray(inputs["bias"], np.float32)

    Whats = []
    for l in range(3):
        if l == 0:
            Fh, one_row, nrm = F0, 136, 136
            rowmap = np.arange(136)
        else:
            Fh, one_row, nrm = FH, 264, 264
            rowmap = np.concatenate([STDCOL, np.arange(256, 264)])
        As = np.zeros((HC, NH), np.float32)
        Ad = np.zeros((HC, NH), np.float32)
        for hh in range(NH):
            As[hh * 32:(hh + 1) * 32, hh] = att_src[l, hh]
            Ad[hh * 32:(hh + 1) * 32, hh] = att_dst[l, hh]
        WAs = W[l] @ As
        WAd = W[l] @ Ad
        What = np.zeros((Fh, CT), np.float32)
        What[:nrm, 0:HC] = W[l][rowmap][:, STDCOL]
        What[one_row, HC:HC + NH] = 1.0
        What[:nrm, 264:272] = WAs[rowmap]
        What[:nrm, 272:280] = WAd[rowmap]
        Whats.append(np.ascontiguousarray(What.astype(bf16)))

    bias_cm = bias[:, STDCOL]
    fin_w1 = np.asarray(inputs["fin_w1"], np.float32)
    fw1 = np.zeros((FH, 528), np.float32)
    fw1[0:256] = fin_w1[STDCOL]
    fw1[256:264] = fin_w1[256:264]
    tmlp_w2_cm = np.ascontiguousarray(np.asarray(inputs["tmlp_w2"], np.float32)[:, STDCOL])
    tmlp_b2_cm = np.ascontiguousarray(np.asarray(inputs["tmlp_b2"], np.float32)[STDCOL])
    return Whats, bias_cm, np.ascontiguousarray(fw1.astype(bf16)), tmlp_w2_cm, tmlp_b2_cm


def host_inputs(inputs, prep):
    n = prep["n"]
    x = np.asarray(inputs["x"], np.float32)
    qY = np.asarray(inputs["q_Y_sample"], np.float32)
    Whats, bias_cm, fw1, tmlp_w2_cm, tmlp_b2_cm = build_weights(inputs)

    slots = prep["slots"]
    real = slots >= 0
    r = np.arange(NSLOT)
    t = r // 128
    k = t % N_CORES
    i = t // N_CORES
    p = r % 128
    phys = k * PB + i * 128 + p

    kk_ = phys[real] // PB
    loc_ = phys[real] % PB
    nodes = slots[real]
    hT = np.zeros((N_CORES, F0, PB), np.float32)
    xq = np.concatenate([x[nodes], qY[nodes]], axis=1)
    hT[kk_[:, None], np.arange(136)[None, :], loc_[:, None]] = xq
    hT[kk_, 136, loc_] = 1.0
    hrepT0 = np.ascontiguousarray(hT.reshape(N_CORES * F0, PB).astype(bf16))

    hq = np.zeros((N_CORES, 16, PB), np.float32)
    hq[kk_[:, None], np.arange(NH)[None, :], loc_[:, None]] = qY[nodes]
    hq[kk_, 8, loc_] = 1.0
    hqT = np.ascontiguousarray(hq.reshape(N_CORES * 16, PB).astype(bf16))

    dumrow = np.zeros((8, CT), np.float32)
    dumrow[:, 264:272] = -1e4
    onepad = np.zeros((128, 8), np.float32)
    onepad[:, 0] = 1.0

    half = 64
    freqs4 = np.exp(np.arange(half, dtype=np.float32)
                    * (-math.log(10000.0) / (half - 1))).astype(np.float32)
    b_repcm = np.stack([np.tile(bias_cm[l][None, :], (128, 1)) for l in range(3)])
    fin_b1 = np.asarray(inputs["fin_b1"], np.float32)
    fin_b2 = np.asarray(inputs["fin_b2"], np.float32)
    b2c = tmlp_b2_cm.reshape(256, 1)

    common = {
        "What0": Whats[0], "What1": Whats[1], "What2": Whats[2],
        "b_repcm": b_repcm.astype(np.float32),
        "fin_w1b": fw1,
        "fin_w2b": np.asarray(inputs["fin_w2"], np.float32).astype(bf16),
        "fin_b1rep": np.tile(fin_b1[None, :], (128, 1)).astype(np.float32),
        "fin_b2rep": np.tile(fin_b2[None, :], (128, 1)).astype(np.float32),
        "tmlp_w1": np.asarray(inputs["tmlp_w1"], np.float32),
        "tmlp_b1col": np.asarray(inputs["tmlp_b1"], np.float32).reshape(128, 1),
        "tmlp_w2cm": tmlp_w2_cm,
        "tmlp_b2cols": np.concatenate([b2c[:128], b2c[128:]], axis=1).astype(np.float32),
        "freqs4": freqs4.reshape(half, 1),
        "t_in": np.asarray(inputs["t"], np.float32).reshape(1, 1),
        "dumrow": dumrow.astype(bf16),
        "onepad": onepad,
        "hrepT0": hrepT0,
        "hqT": hqT,
    }

    in_maps = []
    for kk in range(N_CORES):
        cols = []
        for ii in range(NT):
            for (hf, c0, cc) in prep["plan"][ii]:
                blk = prep["coreA"][kk][ii] if hf == "A" else prep["coreB"][kk][ii]
                cols.append(wrap_idx(blk[c0 * 128:(c0 + cc) * 128]))
        idx_all = np.ascontiguousarray(np.concatenate(cols, axis=1))
        # qY in [128, NT*8] layout: qYs[p, i*8+h] = qY[slot(kk, i, p), h]
        qYs = np.zeros((128, NT * NH), np.float32)
        sel = (k == kk) & real
        qYs[p[sel][:, None], (i[sel] * NH)[:, None] + np.arange(NH)[None, :]] = qY[slots[sel]]
        mA = np.ascontiguousarray(prep["maskA"][kk].astype(bf16))
        mB = np.ascontiguousarray((1.0 - prep["maskA"][kk]).astype(bf16))
        m = dict(common)
        m["qY_shard"] = qYs
        m["idx_all"] = idx_all
        m["maskA"] = mA
        m["maskB"] = mB
        in_maps.append(m)
    return in_maps


# ----------------------------------------------------------------------------
# bass program
# ----------------------------------------------------------------------------
def build_program(prep, ag_engine=AG_ENGINE, chb=None):
    if chb is None:
        chb = CHB
    nch = len(chb) - 1
    plan = prep["plan"]
    IDXC = sum(sum(cc * 8 for (_, _, cc) in plan[ii]) for ii in range(NT))

    nc = bacc.Bacc("TRN2", target_bir_lowering=False, debug=False,
                   enable_asserts=False, num_devices=N_CORES)

    hrepT0 = nc.dram_tensor("hrepT0", [N_CORES * F0, PB], BF16, kind="ExternalInput")
    qYs_d = nc.dram_tensor("qY_shard", [128, NT * NH], F32, kind="ExternalInput")
    idx_all = nc.dram_tensor("idx_all", [128, IDXC], I16, kind="ExternalInput")
    What = [nc.dram_tensor(f"What{l}", [F0 if l == 0 else FH, CT], BF16,
                           kind="ExternalInput") for l in range(3)]
    b_repcm = nc.dram_tensor("b_repcm", [3, 128, HC], F32, kind="ExternalInput")
    fin_w1b = nc.dram_tensor("fin_w1b", [FH, 528], BF16, kind="ExternalInput")
    fin_w2b = nc.dram_tensor("fin_w2b", [528, NH], BF16, kind="ExternalInput")
    fin_b1rep = nc.dram_tensor("fin_b1rep", [128, 528], F32, kind="ExternalInput")
    fin_b2rep = nc.dram_tensor("fin_b2rep", [128, NH], F32, kind="ExternalInput")
    tw1 = nc.dram_tensor("tmlp_w1", [128, 128], F32, kind="ExternalInput")
    tb1c = nc.dram_tensor("tmlp_b1col", [128, 1], F32, kind="ExternalInput")
    tw2 = nc.dram_tensor("tmlp_w2cm", [128, HC], F32, kind="ExternalInput")
    tb2c = nc.dram_tensor("tmlp_b2cols", [128, 2], F32, kind="ExternalInput")
    freqs4 = nc.dram_tensor("freqs4", [64, 1], F32, kind="ExternalInput")
    t_in = nc.dram_tensor("t_in", [1, 1], F32, kind="ExternalInput")
    dumrow = nc.dram_tensor("dumrow", [8, CT], BF16, kind="ExternalInput")
    onepad = nc.dram_tensor("onepad", [128, 8], F32, kind="ExternalInput")
    maskA_d = nc.dram_tensor("maskA", [128, NT], BF16, kind="ExternalInput")
    maskB_d = nc.dram_tensor("maskB", [128, NT], BF16, kind="ExternalInput")
    hqT_d = nc.dram_tensor("hqT", [N_CORES * 16, PB], BF16, kind="ExternalInput")

    out = nc.dram_tensor("out", [L, NH], F32, kind="ExternalOutput")

    T = [nc.dram_tensor(f"table{l}", [NROWS, CT], BF16, kind="Internal")
         for l in range(3)]
    chcols = [(chb[c + 1] - chb[c]) * 128 for c in range(nch)]
    maxcols = max(chcols)
    hTc = {}
    hrepTc = {}
    for l in (1, 2):
        hTc[l] = [nc.dram_tensor(f"hT{l}_{c}", [HC, chcols[c]], BF16, kind="Internal")
                  for c in range(nch)]
        hrepTc[l] = [nc.dram_tensor(f"hrepT{l}_{c}", [N_CORES * HC, chcols[c]], BF16,
                                    kind="Internal", addr_space="Shared")
                     for c in range(nch)]

    def chunk_of(i):
        for c in range(nch):
            if i < chb[c + 1]:
                return c
        raise AssertionError

    with tile.TileContext(nc) as tc:
        import contextlib
        with contextlib.ExitStack() as ctx:
            consts = ctx.enter_context(tc.tile_pool(name="consts", bufs=1))
            sb = ctx.enter_context(tc.tile_pool(name="sb", bufs=3))
            sb2 = ctx.enter_context(tc.tile_pool(name="sb2", bufs=2))
            hfp = ctx.enter_context(tc.tile_pool(name="hfp", bufs=8))
            stp = ctx.enter_context(tc.tile_pool(name="stp", bufs=6))
            gp = ctx.enter_context(tc.tile_pool(name="gp", bufs=4))
            dsb = ctx.enter_context(tc.tile_pool(name="dsb", bufs=2))
            psd = ctx.enter_context(tc.tile_pool(name="psd", bufs=2, space="PSUM"))
            pst = ctx.enter_context(tc.tile_pool(name="pst", bufs=2, space="PSUM"))
            ps1 = ctx.enter_context(tc.tile_pool(name="ps1", bufs=1, space="PSUM"))

            ident = consts.tile([128, 128], F32)
            make_identity(nc, ident[:])

            # ---- temb -> tb[l] [128, 256] f32 (c-major via permuted w2)
            tcol = consts.tile([64, 1], F32, tag="tcol")
            nc.sync.dma_start(out=tcol[0:1, :], in_=t_in[:])
            nc.gpsimd.partition_broadcast(out_ap=tcol[:], in_ap=tcol[0:1, :])
            fq = consts.tile([64, 1], F32, tag="fq")
            nc.sync.dma_start(out=fq[:], in_=freqs4[:])
            xs = consts.tile([64, 1], F32, tag="xs")
            nc.vector.tensor_scalar_mul(xs[:], tcol[:], 4.0)
            ang = consts.tile([64, 1], F32, tag="ang")
            nc.vector.tensor_tensor(out=ang[:], in0=xs[:], in1=fq[:], op=OP.mult)
            TWO_PI = 2 * math.pi
            c1 = float(np.float32(TWO_PI))
            c2 = float(np.float32(TWO_PI - c1))
            c3 = float(TWO_PI - c1 - float(np.float32(TWO_PI - c1)))
            yk = consts.tile([64, 1], F32, tag="yk")
            nc.vector.tensor_scalar_mul(yk[:], ang[:], 1.0 / TWO_PI)
            ki = consts.tile([64, 1], mybir.dt.int32, tag="ki")
            nc.vector.tensor_copy(out=ki[:], in_=yk[:])
            kk_t = consts.tile([64, 1], F32, tag="kk_t")
            nc.vector.tensor_copy(out=kk_t[:], in_=ki[:])
            red = consts.tile([64, 1], F32, tag="red")
            nc.vector.cody_waite_cascade(out=red[:], x=ang[:], k=kk_t[:],
                                         c1=c1, c2=c2, c3=c3)
            rs = consts.tile([64, 1], F32, tag="rs")
            rc = consts.tile([64, 1], F32, tag="rc")
            nc.vector.add_range_wrap(out=rs[:], in_=red[:], shift=0.0,
                                     bound=math.pi, period=TWO_PI)
            nc.vector.add_range_wrap(out=rc[:], in_=red[:], shift=math.pi / 2,
                                     bound=math.pi, period=TWO_PI)
            sc = consts.tile([128, 1], F32, tag="sc")
            sc2 = consts.tile([64, 1], F32, tag="sc2")
            nc.scalar.activation(sc[0:64, :], rs[:], AF.Sin)
            nc.scalar.activation(sc2[:], rc[:], AF.Sin)
            nc.sync.dma_start(out=sc[64:128, :], in_=sc2[:])

            def elu_(xap, tmp_pool, shape, tag):
                # elu(x) = min(exp(x) - 1, relu(x))
                e = tmp_pool.tile(shape, F32, tag=tag + "_e")
                rr = tmp_pool.tile(shape, F32, tag=tag + "_r")
                nc.scalar.activation(e[:], xap, AF.Exp)
                nc.scalar.activation(rr[:], xap, AF.Relu)
                nc.vector.scalar_tensor_tensor(out=xap, in0=e[:], scalar=-1.0,
                                               in1=rr[:], op0=OP.add, op1=OP.min)

            tw1_s = consts.tile([128, 128], F32, tag="tw1")
            nc.sync.dma_start(out=tw1_s[:], in_=tw1[:])
            tw2_s = consts.tile([128, HC], F32, tag="tw2")
            nc.sync.dma_start(out=tw2_s[:], in_=tw2[:])
            e1p = ps1.tile([128, 1], F32, tag="tembp")
            nc.tensor.matmul(out=e1p[:], lhsT=tw1_s[:], rhs=sc[:], start=True, stop=True)
            b1c = consts.tile([128, 1], F32, tag="tb1c")
            nc.sync.dma_start(out=b1c[:], in_=tb1c[:])
            e1 = consts.tile([128, 1], F32, tag="e1")
            nc.vector.tensor_tensor(out=e1[:], in0=e1p[:], in1=b1c[:], op=OP.add)
            elu_(e1[:], consts, [128, 1], "elu_temb")
            tcols_p = ps1.tile([128, 2], F32, tag="tembp")
            nc.tensor.matmul(out=tcols_p[:, 0:1], lhsT=tw2_s[:, 0:128], rhs=e1[:],
                             start=True, stop=True)
            nc.tensor.matmul(out=tcols_p[:, 1:2], lhsT=tw2_s[:, 128:256], rhs=e1[:],
                             start=True, stop=True)
            b2c = consts.tile([128, 2], F32, tag="tb2c")
            nc.sync.dma_start(out=b2c[:], in_=tb2c[:])
            tcols = consts.tile([128, 2], F32, tag="tcols")
            nc.vector.tensor_tensor(out=tcols[:], in0=tcols_p[:], in1=b2c[:], op=OP.add)
            trow_p = ps1.tile([2, 128], F32, tag="tembp")
            nc.tensor.transpose(out=trow_p[:], in_=tcols[:], identity=ident[:])
            trow2 = consts.tile([2, 128], F32, tag="trow2")
            nc.scalar.copy(out=trow2[:], in_=trow_p[:])
            trow = consts.tile([1, HC], F32, tag="trow")
            nc.sync.dma_start(out=trow[0:1, 0:128], in_=trow2[0:1, :])
            nc.sync.dma_start(out=trow[0:1, 128:256], in_=trow2[1:2, :])
            temb_rep = consts.tile([128, HC], F32, tag="temb_rep")
            nc.gpsimd.partition_broadcast(out_ap=temb_rep[:], in_ap=trow[:])
            tb = []
            for l in range(3):
                bl = consts.tile([128, HC], F32, tag=f"b_rep{l}")
                nc.sync.dma_start(out=bl[:], in_=b_repcm[l])
                tbl = consts.tile([128, HC], F32, tag=f"tb{l}")
                nc.vector.tensor_tensor(out=tbl[:], in0=temb_rep[:], in1=bl[:], op=OP.add)
                tb.append(tbl)

            # ---- weights
            Wch = []
            for l in range(3):
                F = F0 if l == 0 else FH
                cks = []
                off = 0
                while off < F:
                    kk = min(128, F - off)
                    wt = consts.tile([128, CT], BF16, tag=f"W{l}_{off}")
                    nc.sync.dma_start(out=wt[:kk, :], in_=What[l][off:off + kk, :])
                    cks.append((wt, kk))
                    off += kk
                Wch.append(cks)
            fw1t = []
            off = 0
            while off < FH:
                kk = min(128, FH - off)
                wt = consts.tile([128, 528], BF16, tag=f"fw1_{off}")
                nc.sync.dma_start(out=wt[:kk, :], in_=fin_w1b[off:off + kk, :])
                fw1t.append((wt, kk))
                off += kk
            fw2t = []
            off = 0
            while off < 528:
                kk = min(128, 528 - off)
                wt = consts.tile([128, NH], BF16, tag=f"fw2_{off}")
                nc.sync.dma_start(out=wt[:kk, :], in_=fin_w2b[off:off + kk, :])
                fw2t.append((wt, kk))
                off += kk
            fb1 = consts.tile([128, 528], F32, tag="fb1")
            nc.sync.dma_start(out=fb1[:], in_=fin_b1rep[:])
            fb2 = consts.tile([128, NH], F32, tag="fb2")
            nc.sync.dma_start(out=fb2[:], in_=fin_b2rep[:])
            onep = consts.tile([128, 8], F32, tag="onep")
            nc.sync.dma_start(out=onep[:], in_=onepad[:])
            dum_t = consts.tile([8, CT], BF16, tag="dum")
            nc.sync.dma_start(out=dum_t[:], in_=dumrow[:])
            mA_s = consts.tile([128, NT], BF16, tag="mA")
            nc.sync.dma_start(out=mA_s[:], in_=maskA_d[:])
            mB_s = consts.tile([128, NT], BF16, tag="mB")
            nc.sync.dma_start(out=mB_s[:], in_=maskB_d[:])
            qYs_s = consts.tile([128, NT * NH], F32, tag="qYs")
            nc.sync.dma_start(out=qYs_s[:], in_=qYs_d[:])
            idx_s = consts.tile([128, IDXC], I16, tag="idx_s")
            nc.sync.dma_start(out=idx_s[:], in_=idx_all[:])

            # ---- dense helpers (batched table writes, 4 tiles per DMA)
            def dense_range(l, kk_blk, lhs_strips, tiles, col_of_tile):
                group = []

                def flush():
                    if not group:
                        return
                    g = len(group)
                    i0 = group[0][1]
                    Ts4 = dsb.tile([128, 2, CW], BF16, tag="Ts4")
                    for (j, (pT, _)) in enumerate(group):
                        nc.scalar.copy(out=Ts4[:, j, :], in_=pT[:])
                    dst = T[l][kk_blk * PB + i0 * 128:
                               kk_blk * PB + (i0 + g) * 128, 0:CW]
                    nc.sync.dma_start(
                        out=dst.rearrange("(t p) c -> p t c", t=g),
                        in_=Ts4[:, 0:g, :])
                    group.clear()

                for i in tiles:
                    c0 = col_of_tile(i)
                    pT = psd.tile([128, CW], F32, tag="pT")
                    ncks = len(lhs_strips)
                    for ci, (st, kk) in enumerate(lhs_strips):
                        nc.tensor.matmul(out=pT[:],
                                         lhsT=st[:kk, c0:c0 + 128],
                                         rhs=Wch[l][ci][0][:kk, 0:CW],
                                         start=(ci == 0), stop=(ci == ncks - 1))
                    group.append((pT, i))
                    if len(group) == 2:
                        flush()
                flush()

            def dense_chunk(l, ch):
                t0, t1 = chb[ch], chb[ch + 1]
                for sub0 in range(t0, t1, 16):
                    sub1 = min(sub0 + 16, t1)
                    cols = (sub1 - sub0) * 128
                    for kk_blk in range(N_CORES):
                        if l == 0:
                            pieces = [(hrepT0, kk_blk * F0, 128, sub0 * 128),
                                      (hrepT0, kk_blk * F0 + 128, 16, sub0 * 128)]
                        else:
                            pieces = [(hrepTc[l][ch], kk_blk * HC, 128,
                                       (sub0 - t0) * 128),
                                      (hrepTc[l][ch], kk_blk * HC + 128, 128,
                                       (sub0 - t0) * 128),
                                      (hqT_d, kk_blk * 16, 16, sub0 * 128)]
                        strips = []
                        for pi, (srct, r0, kk, csrc) in enumerate(pieces):
                            st = dsb.tile([128, 2048], BF16, tag=f"ds{pi}")
                            nc.sync.dma_start(
                                out=st[:kk, 0:cols],
                                in_=srct[r0:r0 + kk, csrc:csrc + cols])
                            strips.append((st, kk))
                        dense_range(l, kk_blk, strips, range(sub0, sub1),
                                    lambda i: (i - sub0) * 128)

            def dumfix(l):
                for kk_blk in range(N_CORES):
                    nc.sync.dma_start(out=T[l][kk_blk * PB + L:kk_blk * PB + PB, :],
                                      in_=dum_t[:])

            # ---- edge phase
            idx_off_by_tile = []
            off = 0
            for ii in range(NT):
                idx_off_by_tile.append(off)
                off += sum(cc * 8 for (_, _, cc) in plan[ii])

            def ag_issue(lnext, ch):
                eng = nc.gpsimd if ag_engine == "pool" else nc.tensor
                cbass.BassGpSimd.collective_compute(
                    eng, "AllGather", OP.bypass,
                    replica_groups=[list(range(N_CORES))],
                    ins=[hTc[lnext][ch][:]], outs=[hrepTc[lnext][ch][:]])

            def edge_tile(l, i):
                TA = T[l][0:HALF, :]
                TB = T[l][HALF:NROWS, :]
                ioff = idx_off_by_tile[i]
                # gather all chunks up-front (plan order), track first A / first B
                gts = []
                co = ioff
                for (hf, c0, cc) in plan[i]:
                    tbl = TA if hf == "A" else TB
                    g_t = gp.tile([128, CMAX, CT], BF16, tag="g")
                    nc.gpsimd.dma_gather(out_ap=g_t[:, 0:cc, :], in_ap=tbl,
                                         idxs_ap=idx_s[:, co:co + cc * 8],
                                         num_idxs=128 * cc, num_idxs_reg=128 * cc,
                                         elem_size=CT, single_packet=False)
                    co += cc * 8
                    gts.append((hf, c0, cc, g_t))
                gA0 = next(g for (hf, c0, cc, g) in gts if hf == "A" and c0 == 0)
                gB0 = next(g for (hf, c0, cc, g) in gts if hf == "B" and c0 == 0)
                # own alpha_dst from self-loop slots (col 0 of own half)
                ad_t = sb.tile([128, NH], BF16, tag="ad")
                adb = sb.tile([128, NH], BF16, tag="adb")
                nc.vector.tensor_tensor(
                    out=ad_t[:], in0=gA0[:, 0, 272:280],
                    in1=mA_s[:, i:i + 1].broadcast_to([128, NH]), op=OP.mult)
                nc.vector.tensor_tensor(
                    out=adb[:], in0=gB0[:, 0, 272:280],
                    in1=mB_s[:, i:i + 1].broadcast_to([128, NH]), op=OP.mult)
                nc.vector.tensor_tensor(out=ad_t[:], in0=ad_t[:], in1=adb[:],
                                        op=OP.add)

                acc = sb.tile([128, 264], F32, tag="acc")
                # logits -> exp for all chunks first (hides ACT latency
                # behind the other chunks' DVE work)
                wts = []
                for (hf, c0, cc, g_t) in gts:
                    lg = sb.tile([128, CMAX, NH], BF16, tag="lg")
                    nc.vector.tensor_tensor(
                        out=lg[:, 0:cc, :], in0=g_t[:, 0:cc, 264:272],
                        in1=ad_t[:].unsqueeze(1).broadcast_to([128, cc, NH]),
                        op=OP.add)
                    nc.vector.scalar_tensor_tensor(
                        out=lg[:, 0:cc, :], in0=lg[:, 0:cc, :], scalar=0.2,
                        in1=lg[:, 0:cc, :], op0=OP.mult, op1=OP.max)
                    w_t = sb.tile([128, CMAX, NH], BF16, tag="w")
                    nc.scalar.activation(w_t[:, 0:cc, :], lg[:, 0:cc, :], AF.Exp)
                    wts.append(w_t)
                first = True
                for (hf, c0, cc, g_t), w_t in zip(gts, wts):
                    tmp = sb2.tile([128, CMAX, 264], BF16, tag="tmp")
                    gv = g_t[:, 0:cc, 0:264].rearrange("p j (c h) -> p j c h", c=33)
                    wv = w_t[:, 0:cc, :].unsqueeze(2).broadcast_to([128, cc, 33, NH])
                    tv = tmp[:, 0:cc, :].rearrange("p j (c h) -> p j c h", c=33)
                    nc.vector.tensor_tensor(out=tv, in0=gv, in1=wv, op=OP.mult)
                    nn = cc
                    while nn > 2:
                        a = nn // 2
                        nc.vector.tensor_tensor(out=tmp[:, 0:a, :],
                                                in0=tmp[:, 0:a, :],
                                                in1=tmp[:, a:2 * a, :], op=OP.add)
                        if nn % 2:
                            nc.vector.tensor_tensor(out=tmp[:, 0:1, :],
                                                    in0=tmp[:, 0:1, :],
                                                    in1=tmp[:, 2 * a:2 * a + 1, :],
                                                    op=OP.add)
                        nn = a
                    if first:
                        if nn == 2:
                            nc.vector.tensor_tensor(out=acc[:], in0=tmp[:, 0, :],
                                                    in1=tmp[:, 1, :], op=OP.add)
                        else:
                            nc.vector.tensor_copy(out=acc[:], in_=tmp[:, 0, :])
                        first = False
                    else:
                        part = sb.tile([128, 264], F32, tag="part")
                        if nn == 2:
                            nc.vector.tensor_tensor(out=part[:], in0=tmp[:, 0, :],
                                                    in1=tmp[:, 1, :], op=OP.add)
                        else:
                            nc.vector.tensor_copy(out=part[:], in_=tmp[:, 0, :])
                        nc.vector.tensor_tensor(out=acc[:], in0=acc[:], in1=part[:],
                                                op=OP.add)

                rcp = sb.tile([128, NH], F32, tag="rcp")
                nc.vector.reciprocal(rcp[:], acc[:, 256:264])
                hfull = hfp.tile([128, FH], F32, tag="hfull")
                nc.vector.tensor_tensor(
                    out=hfull[:, 0:256].rearrange("p (c h) -> p c h", c=32),
                    in0=acc[:, 0:256].rearrange("p (c h) -> p c h", c=32),
                    in1=rcp[:].unsqueeze(1).broadcast_to([128, 32, NH]),
                    op=OP.mult)
                nc.vector.tensor_tensor(out=hfull[:, 0:256], in0=hfull[:, 0:256],
                                        in1=tb[l][:], op=OP.add)
                elu_(hfull[:, 0:256], sb, [128, 256], "eluh")
                if l == 2:
                    nc.scalar.copy(out=hfull[:, 256:264],
                                   in_=qYs_s[:, i * NH:(i + 1) * NH])
                    nc.scalar.copy(out=hfull[:, 264:272], in_=onep[:])
                if l < 2:
                    ch = chunk_of(i)
                    col0 = (i - chb[ch]) * 128
                    for (offr, kk) in ((0, 128), (128, 128)):
                        pt = pst.tile([128, 128], F32, tag="pt")
                        nc.tensor.transpose(out=pt[:kk, :],
                                            in_=hfull[:, offr:offr + kk],
                                            identity=ident[:])
                        st = stp.tile([128, 128], BF16, tag=f"st{offr}")
                        nc.scalar.copy(out=st[:kk, :], in_=pt[:kk, :])
                        nc.sync.dma_start(
                            out=hTc[l + 1][ch][offr:offr + kk, col0:col0 + 128],
                            in_=st[:kk, :])
                else:
                    hts = []
                    for (offr, kk) in ((0, 128), (128, 128), (256, 16)):
                        pt = pst.tile([128, 128], F32, tag="pt")
                        nc.tensor.transpose(out=pt[:kk, :],
                                            in_=hfull[:, offr:offr + kk],
                                            identity=ident[:])
                        st = stp.tile([128, 128], BF16, tag=f"st{offr}")
                        nc.scalar.copy(out=st[:kk, :], in_=pt[:kk, :])
                        hts.append((st, kk))
                    u = sb.tile([128, 528], F32, tag="u")
                    for half_i in range(2):
                        pm = ps1.tile([128, 264], F32, tag="pmlp")
                        for ci, (st, kk) in enumerate(hts):
                            nc.tensor.matmul(
                                out=pm[:], lhsT=st[:kk, :],
                                rhs=fw1t[ci][0][:kk, half_i * 264:(half_i + 1) * 264],
                                start=(ci == 0), stop=(ci == 2))
                        nc.vector.tensor_tensor(
                            out=u[:, half_i * 264:(half_i + 1) * 264],
                            in0=pm[:], in1=fb1[:, half_i * 264:(half_i + 1) * 264],
                            op=OP.add)
                    elu_(u[:], sb, [128, 528], "elu_u")
                    po = ps1.tile([128, NH], F32, tag="po")
                    for ci in range(5):
                        offc = ci * 128
                        kk = min(128, 528 - offc)
                        pt = pst.tile([128, 128], F32, tag="pt")
                        nc.tensor.transpose(out=pt[:kk, :],
                                            in_=u[:, offc:offc + kk],
                                            identity=ident[:])
                        st = stp.tile([128, 128], BF16, tag="uT")
                        nc.scalar.copy(out=st[:kk, :], in_=pt[:kk, :])
                        nc.tensor.matmul(out=po[:], lhsT=st[:kk, :],
                                         rhs=fw2t[ci][0][:kk, :],
                                         start=(ci == 0), stop=(ci == 4))
                    o_t = sb.tile([128, NH], F32, tag="o_t")
                    nc.vector.tensor_tensor(out=o_t[:], in0=po[:], in1=fb2[:],
                                            op=OP.add)
                    nc.sync.dma_start(out=out[i * 128:(i + 1) * 128, :], in_=o_t[:])

            # ---- schedule
            for ch in range(nch):
                dense_chunk(0, ch)
            dumfix(0)
            for l in range(3):
                for i in range(NT):
                    edge_tile(l, i)
                    if l < 2 and i + 1 in chb[1:]:
                        ch = chb[1:].index(i + 1)
                        ag_issue(l + 1, ch)
                        dense_chunk(l + 1, ch)
                if l < 2:
                    dumfix(l + 1)

    nc.compile()
    return nc


def run(inputs, trace=False):
    from concourse.bass_utils import run_bass_kernel_spmd
    from concourse.bass_interp import get_hw_module
    adj = np.asarray(inputs["adj"])
    n = int(np.asarray(inputs["x"]).shape[0])
    prep = preprocess(adj, n)
    in_maps = host_inputs(inputs, prep)
    nc = build_program(prep)
    nc.m = get_hw_module(nc.m)
    res = run_bass_kernel_spmd(nc, in_maps, core_ids=list(range(N_CORES)),
                               trace=trace)
    outs = [np.asarray(r["out"]) for r in res.results]
    y_slots = np.zeros((NSLOT, NH), np.float32)
    for k in range(N_CORES):
        for i in range(NT):
            slot_base = (i * N_CORES + k) * 128
            y_slots[slot_base:slot_base + 128] = outs[k][i * 128:(i + 1) * 128]
    slots = prep["slots"]
    r_real = np.flatnonzero(slots >= 0)
    y = np.zeros((n, NH), np.float32)
    y[slots[r_real]] = y_slots[r_real]
    return y, res


def kernel(**inputs) -> np.ndarray:
    y, _ = run(inputs)
    return y
